# revision 30
# baseline (speedup 1.0000x reference)
"""Trainium2 Bass kernel for nn_CA_80461917323389 (sparse_attention).

Reference computation (per batch b, one NeuronCore per batch):
  xt  = LN(xf)                                   [N=256, TXT=768]
  q   = softmax((LN(x) @ Wq + bq).view(T,H,64))  [T=8192, H=8, 64]
  k   = softmax((xt @ Wk + bk).view(N,H,64))
  v   = (xt @ Wv + bv).view(N,H,64)
  attn[h] = k[:,h,:].T @ v[:,h,:]                [H, 64, 64]
  out = q @ attn (per head)                      [T, 512]
  eo  = silu(emb) @ emb_W + emb_b ; scale, shift = split(eo)
  h   = LN(out) * (1+scale) + shift
  y   = x + silu(h) @ out_W + out_b

Sharding: data-parallel over B=8 across the 8 cores.

Host-side prep is weights-only folding:
  - LN gains/biases folded into Wq/Wk/Wv (g[:,None]*W, b@W+bias)
  - silu(z) = (tanh(z/2)+1) * z * 0.5 -> the 0.5 is folded into out_W and
    emb_W so ScalarE only ever needs the exp_and_others table set (exp+tanh).
"""

import os
import sys

import numpy as np

sys.path.insert(0, "/opt/trn_rl_repo")

import ml_dtypes  # noqa: E402

BF16 = ml_dtypes.bfloat16

B, T, N, D, TXT, TE, H = 8, 8192, 256, 512, 768, 2048, 8
DH = D // H  # 64
P = 128
KC = D // P    # 4 k-chunks for D
KCT = TXT // P  # 6 k-chunks for TXT
EPS = 1e-5
RSQRT_MAGIC = 0x5F3759DF


def _rsqrt_chain(nc, pool, var_ap, eps, n_newton=1):
    """1/sqrt(var + eps) on VectorE only (no ACT table dependency).

    Quake-III bit trick init + Newton iterations. All ops on [P, w] tiles.
    """
    import concourse.mybir as mybir

    shape = list(var_ap.shape)
    alu = mybir.AluOpType
    vp = pool.tile(shape, mybir.dt.float32, tag="ch_vp")
    nc.vector.tensor_scalar(out=vp, in0=var_ap, scalar1=float(eps), scalar2=None,
                            op0=alu.add)
    y = pool.tile(shape, mybir.dt.float32, tag="ch_y")
    vi = vp.bitcast(mybir.dt.int32)
    yi = y.bitcast(mybir.dt.int32)
    # yi = MAGIC - (vi >> 1)
    nc.vector.tensor_scalar(out=yi, in0=vi, scalar1=1, scalar2=None,
                            op0=alu.logical_shift_right)
    nc.vector.tensor_scalar(out=yi, in0=yi, scalar1=-1, scalar2=RSQRT_MAGIC,
                            op0=alu.mult, op1=alu.add)
    t1 = pool.tile(shape, mybir.dt.float32, tag="ch_t1")
    for _ in range(n_newton):
        # y <- y * (1.5 - 0.5 * vp * y*y)
        nc.vector.tensor_tensor(out=t1, in0=y, in1=y, op=alu.mult)
        nc.vector.tensor_tensor(out=t1, in0=t1, in1=vp, op=alu.mult)
        nc.vector.tensor_scalar(out=t1, in0=t1, scalar1=-0.5, scalar2=1.5,
                                op0=alu.mult, op1=alu.add)
        nc.vector.tensor_tensor(out=y, in0=y, in1=t1, op=alu.mult)
    return y


def build_program(n_token_tiles=T // P, repeat=1):
    """Build the Bass program (shared by all 8 cores, SPMD).

    repeat>1 wraps the main token loop in a device-side For_i that redoes
    the identical work `repeat` times — used only for timing amplification.
    """
    import contextlib
    from contextlib import ExitStack

    import concourse.bacc as bacc
    import concourse.bass as bass
    import concourse.mybir as mybir
    import concourse.tile as tile
    from concourse.masks import make_identity

    f32 = mybir.dt.float32
    bf16 = mybir.dt.bfloat16
    alu = mybir.AluOpType
    act = mybir.ActivationFunctionType

    TT = n_token_tiles  # token tiles of 128 rows

    nc = bacc.Bacc("TRN2", target_bir_lowering=False, debug=False)
    x_d = nc.dram_tensor("x", [TT * P, D], bf16, kind="ExternalInput")
    xf_d = nc.dram_tensor("xf", [N, TXT], f32, kind="ExternalInput")
    embt_d = nc.dram_tensor("embt", [P, TE // P], f32, kind="ExternalInput")
    wq_d = nc.dram_tensor("wq", [D, D], bf16, kind="ExternalInput")
    wk_d = nc.dram_tensor("wk", [TXT, D], bf16, kind="ExternalInput")
    wv_d = nc.dram_tensor("wv", [TXT, D], bf16, kind="ExternalInput")
    wo_d = nc.dram_tensor("wo", [D, D], bf16, kind="ExternalInput")
    wemb_d = nc.dram_tensor("wemb", [TE, 2 * D], bf16, kind="ExternalInput")
    go_d = nc.dram_tensor("go", [1, D], f32, kind="ExternalInput")
    bo_d = nc.dram_tensor("bo", [1, D], f32, kind="ExternalInput")
    embb_d = nc.dram_tensor("embb", [1, 2 * D], f32, kind="ExternalInput")
    y_d = nc.dram_tensor("y", [TT * P, D], bf16, kind="ExternalOutput")

    with tile.TileContext(nc) as tc, ExitStack() as ctx:
        const = ctx.enter_context(tc.tile_pool(name="const", bufs=1))

        # ---- constants / weights into SBUF ----
        ident = const.tile([P, P], bf16)
        make_identity(nc, ident)
        ones_f32 = const.tile([1, P], f32)
        nc.vector.memset(ones_f32, 1.0)

        wq_sb = const.tile([P, KC, D], bf16)
        nc.sync.dma_start(out=wq_sb, in_=wq_d.rearrange("(c p) n -> p c n", p=P))
        wk_sb = const.tile([P, KCT, D], bf16)
        nc.sync.dma_start(out=wk_sb, in_=wk_d.rearrange("(c p) n -> p c n", p=P))
        wv_sb = const.tile([P, KCT, D], bf16)
        nc.sync.dma_start(out=wv_sb, in_=wv_d.rearrange("(c p) n -> p c n", p=P))
        wo_sb = const.tile([P, KC, D], bf16)
        nc.sync.dma_start(out=wo_sb, in_=wo_d.rearrange("(c p) n -> p c n", p=P))
        wemb_sb = const.tile([P, TE // P, 2 * D], bf16)
        nc.sync.dma_start(out=wemb_sb, in_=wemb_d.rearrange("(c p) n -> p c n", p=P))
        go_sb = const.tile([1, D], f32)
        nc.sync.dma_start(out=go_sb, in_=go_d[:, :])
        bo_sb = const.tile([1, D], f32)
        nc.sync.dma_start(out=bo_sb, in_=bo_d[:, :])
        embb_sb = const.tile([1, 2 * D], f32)
        nc.sync.dma_start(out=embb_sb, in_=embb_d[:, :])

        scale_rep = const.tile([P, D], bf16)   # (1+scale)*g_o, replicated
        shift_rep = const.tile([P, D], bf16)   # b_o*(1+scale)+shift, replicated
        a_sb = const.tile([P, KC, DH * 2 + 2], bf16)  # head-pair blockdiag + sum cols

        small = ctx.enter_context(tc.tile_pool(name="small", bufs=int(os.environ.get("KBUF_SMALL", 12))))

        # =================== prologue: eo -> scale/shift ===================
        with tc.tile_pool(name="pro_eo", bufs=2) as pro, \
             tc.tile_pool(name="pro_eo_ps", bufs=1, space="PSUM") as pro_ps:
            embt = pro.tile([P, TE // P], f32)
            nc.sync.dma_start(out=embt, in_=embt_d[:, :])
            th_e = pro.tile([P, TE // P], f32)
            nc.scalar.activation(out=th_e, in_=embt, func=act.Tanh, scale=0.5)
            se = pro.tile([P, TE // P], bf16)
            th_p1 = pro.tile([P, TE // P], f32)
            nc.vector.tensor_scalar(out=th_p1, in0=th_e, scalar1=1.0,
                                    scalar2=None, op0=alu.add)
            nc.vector.tensor_tensor(out=se, in0=th_p1, in1=embt, op=alu.mult)
            ps_sc = pro_ps.tile([1, D], f32)
            ps_sh = pro_ps.tile([1, D], f32)
            nkc = TE // P
            for kc in range(nkc):
                nc.tensor.matmul(ps_sc, lhsT=se[:, kc : kc + 1],
                                 rhs=wemb_sb[:, kc, 0:D],
                                 start=(kc == 0), stop=(kc == nkc - 1))
            for kc in range(nkc):
                nc.tensor.matmul(ps_sh, lhsT=se[:, kc : kc + 1],
                                 rhs=wemb_sb[:, kc, D : 2 * D],
                                 start=(kc == 0), stop=(kc == nkc - 1))
            # sp1 = (scale + emb_b[:D]) + 1
            sp1 = pro.tile([1, D], f32)
            nc.vector.scalar_tensor_tensor(out=sp1, in0=ps_sc, scalar=1.0,
                                           in1=embb_sb[:, 0:D],
                                           op0=alu.add, op1=alu.add)
            scale_row = pro.tile([1, D], f32)
            nc.vector.tensor_tensor(out=scale_row, in0=sp1, in1=go_sb, op=alu.mult)
            # shift_row = (shift + emb_b[D:]) + b_o * sp1
            t_bo = pro.tile([1, D], f32)
            nc.vector.tensor_tensor(out=t_bo, in0=sp1, in1=bo_sb, op=alu.mult)
            shift_row = pro.tile([1, D], f32)
            nc.vector.scalar_tensor_tensor(out=shift_row, in0=ps_sh, scalar=0.0,
                                           in1=embb_sb[:, D : 2 * D],
                                           op0=alu.add, op1=alu.add)
            nc.vector.tensor_tensor(out=shift_row, in0=shift_row, in1=t_bo,
                                    op=alu.add)
            # broadcast rows across 128 partitions via ones-matmul
            ps_bc = pro_ps.tile([P, D], f32, tag="bc")
            nc.tensor.matmul(ps_bc, lhsT=ones_f32, rhs=scale_row,
                             start=True, stop=True)
            nc.scalar.copy(out=scale_rep, in_=ps_bc)
            ps_bc2 = pro_ps.tile([P, D], f32, tag="bc")
            nc.tensor.matmul(ps_bc2, lhsT=ones_f32, rhs=shift_row,
                             start=True, stop=True)
            nc.scalar.copy(out=shift_rep, in_=ps_bc2)

        # =================== prologue: k/v -> attn pairs ===================
        with tc.tile_pool(name="pro_kv", bufs=2) as kvp, \
             tc.tile_pool(name="pro_kv_ps", bufs=1, space="PSUM") as kv_ps, \
             tc.tile_pool(name="pro_a_ps", bufs=4, space="PSUM") as a_ps:
            NTILES = N // P  # 2
            k_n = [None] * NTILES
            v_b = [None] * NTILES
            for tt in range(NTILES):
                xf_sb = kvp.tile([P, TXT], f32, tag="xf")
                nc.sync.dma_start(out=xf_sb, in_=xf_d[tt * P : (tt + 1) * P, :])
                st = kvp.tile([P, 3, 6], f32, tag="st")
                xf_g = xf_sb.rearrange("p (g d) -> p g d", g=3)
                for g in range(3):
                    nc.vector.bn_stats(out=st[:, g, :], in_=xf_g[:, g, :])
                mv = kvp.tile([P, 2], f32, tag="mv")
                nc.vector.bn_aggr(out=mv, in_=st)
                inv_t = _rsqrt_chain(nc, small, mv[:, 1:2], EPS)
                xtn = kvp.tile([P, TXT], bf16, tag="xtn")
                nc.vector.tensor_scalar(out=xtn, in0=xf_sb, scalar1=mv[:, 0:1],
                                        scalar2=inv_t, op0=alu.subtract,
                                        op1=alu.mult)
                xtT = kvp.tile([P, KCT, P], bf16, tag="xtT")
                nc.sync.dma_start_transpose(out=xtT, in_=xtn)

                ps_k = kv_ps.tile([P, D], f32, tag="psk")
                for c in range(KCT):
                    nc.tensor.matmul(ps_k, lhsT=xtT[:, c, :], rhs=wk_sb[:, c, :],
                                     start=(c == 0), stop=(c == KCT - 1))
                k_e = kvp.tile([P, D], bf16, tag="ke")
                nc.scalar.activation(out=k_e, in_=ps_k, func=act.Exp)
                ks = kvp.tile([P, H], f32, tag="ks")
                nc.vector.tensor_reduce(out=ks, in_=k_e.rearrange(
                    "p (h d) -> p h d", h=H), axis=mybir.AxisListType.X,
                    op=alu.add)
                kr = kvp.tile([P, H], f32, tag="kr")
                nc.vector.reciprocal(out=kr, in_=ks)
                k_n[tt] = kvp.tile([P, D], bf16, tag=f"kn{tt}", name=f"kn{tt}")
                nc.vector.tensor_tensor(
                    out=k_n[tt].rearrange("p (h d) -> p h d", h=H),
                    in0=k_e.rearrange("p (h d) -> p h d", h=H),
                    in1=kr.unsqueeze(2).broadcast_to([P, H, DH]), op=alu.mult)

                ps_v = kv_ps.tile([P, D], f32, tag="psv")
                for c in range(KCT):
                    nc.tensor.matmul(ps_v, lhsT=xtT[:, c, :], rhs=wv_sb[:, c, :],
                                     start=(c == 0), stop=(c == KCT - 1))
                v_b[tt] = kvp.tile([P, D], bf16, tag=f"vb{tt}", name=f"vb{tt}")
                nc.scalar.copy(out=v_b[tt], in_=ps_v)

            # attn[h] = k[:,h].T @ v[:,h], assembled as head-pair blockdiag
            nc.vector.memset(a_sb, 0.0)
            for c in range(KC):
                ps_a = a_ps.tile([P, P], f32)
                for tt in range(NTILES):
                    h0 = 2 * c
                    nc.tensor.matmul(
                        ps_a[0:DH, 0:DH],
                        lhsT=k_n[tt][:, h0 * DH : (h0 + 1) * DH],
                        rhs=v_b[tt][:, h0 * DH : (h0 + 1) * DH],
                        start=(tt == 0), stop=(tt == NTILES - 1))
                for tt in range(NTILES):
                    h1 = 2 * c + 1
                    nc.tensor.matmul(
                        ps_a[DH : 2 * DH, DH : 2 * DH],
                        lhsT=k_n[tt][:, h1 * DH : (h1 + 1) * DH],
                        rhs=v_b[tt][:, h1 * DH : (h1 + 1) * DH],
                        start=(tt == 0), stop=(tt == NTILES - 1),
                        tile_position=(0, 64))
                nc.vector.tensor_copy(out=a_sb[0:DH, c, 0:DH],
                                      in_=ps_a[0:DH, 0:DH])
                nc.vector.tensor_copy(out=a_sb[DH : 2 * DH, c, DH : 2 * DH],
                                      in_=ps_a[DH : 2 * DH, DH : 2 * DH])
            nc.vector.memset(a_sb[0:DH, :, 2 * DH : 2 * DH + 1], 1.0)
            nc.vector.memset(a_sb[DH : 2 * DH, :, 2 * DH + 1 : 2 * DH + 2], 1.0)

        # =================== main loop over token tiles ===================
        stream = ctx.enter_context(tc.tile_pool(name="stream", bufs=int(os.environ.get("KBUF_STREAM", 7))))
        work = ctx.enter_context(tc.tile_pool(name="work", bufs=int(os.environ.get("KBUF_WORK", 7))))
        ps_qT_p = ctx.enter_context(tc.tile_pool(name="ps_qT", bufs=2, space="PSUM"))
        ps_o_p = ctx.enter_context(tc.tile_pool(name="ps_o", bufs=2, space="PSUM"))
        ps_s_p = ctx.enter_context(tc.tile_pool(name="ps_s", bufs=1, space="PSUM"))
        ps_y_p = ctx.enter_context(tc.tile_pool(name="ps_y", bufs=1, space="PSUM"))
        ps_t_p = ctx.enter_context(tc.tile_pool(name="ps_t", bufs=2, space="PSUM"))

        rep_cm = tc.For_i(0, repeat, 1) if repeat > 1 else contextlib.nullcontext()

        def stage_a(it):
            """load + LN1 + transpose -> returns state for stage b"""
            r0 = it * P
            xt_f = stream.tile([P, D], bf16, tag="x_in", name=f"x_{it}")
            nc.sync.dma_start(out=xt_f, in_=x_d[r0 : r0 + P, :])
            st6 = work.tile([P, 6], f32, tag="st6", name=f"st6_{it}")
            nc.vector.bn_stats(out=st6, in_=xt_f)
            mv = work.tile([P, 2], f32, tag="mv", name=f"mv_{it}")
            nc.vector.bn_aggr(out=mv, in_=st6)
            inv1 = _rsqrt_chain(nc, small, mv[:, 1:2], EPS, n_newton=0)
            xn_b = work.tile([P, D], bf16, tag="xn", name=f"xn_{it}")
            nc.vector.tensor_scalar(out=xn_b, in0=xt_f, scalar1=mv[:, 0:1],
                                    scalar2=inv1, op0=alu.subtract,
                                    op1=alu.mult)
            xT = work.tile([P, KC, P], bf16, tag="xT", name=f"xT_{it}")
            nc.sync.dma_start_transpose(out=xT, in_=xn_b)
            return {"x": xt_f, "xT": xT}

        def stage_b1(st, it):
            """q projection + exp"""
            ps_qT = ps_qT_p.tile([P, KC, P], f32, tag="psqT", name=f"psqT_{it}")
            for dc in range(KC):
                for kc in range(KC):
                    nc.tensor.matmul(ps_qT[:, dc, :],
                                     lhsT=wq_sb[:, kc, dc * P : (dc + 1) * P],
                                     rhs=st["xT"][:, kc, :],
                                     start=(kc == 0), stop=(kc == KC - 1))
            q_eT = work.tile([P, KC, P], bf16, tag="qeT", name=f"qeT_{it}")
            nc.scalar.activation(out=q_eT, in_=ps_qT, func=act.Exp)
            st["qeT"] = q_eT

        def stage_b2(st, it):
            """attention apply + softmax div + LN2 scalars"""
            q_eT = st.pop("qeT")
            ps_o = ps_o_p.tile([P, D], f32, tag="pso", name=f"pso_{it}")
            ps_s = ps_s_p.tile([P, H], f32, tag="pss", name=f"pss_{it}")
            for c in range(KC):
                nc.tensor.matmul(ps_o[:, c * P : (c + 1) * P],
                                 lhsT=q_eT[:, c, :], rhs=a_sb[:, c, 0 : 2 * DH],
                                 start=True, stop=True)
                nc.tensor.matmul(ps_s[:, 2 * c : 2 * c + 2],
                                 lhsT=q_eT[:, c, :],
                                 rhs=a_sb[:, c, 2 * DH : 2 * DH + 2],
                                 start=True, stop=True)
            r = work.tile([P, H], f32, tag="r", name=f"r_{it}")
            nc.vector.reciprocal(out=r, in_=ps_s)
            od = work.tile([P, D], bf16, tag="od", name=f"od_{it}")
            s1 = work.tile([P, 1], f32, tag="s1", name=f"s1_{it}")
            nc.vector.scalar_tensor_tensor(
                out=od.rearrange("p (h d) -> p h d", h=H),
                in0=ps_o.rearrange("p (h d) -> p h d", h=H), scalar=1.0,
                in1=r.unsqueeze(2).broadcast_to([P, H, DH]),
                op0=alu.mult, op1=alu.mult, accum_out=s1)
            junk = work.tile([P, D], bf16, tag="junk", name=f"junk_{it}")
            s2 = work.tile([P, 1], f32, tag="s2", name=f"s2_{it}")
            nc.scalar.activation(out=junk, in_=od, func=act.Square,
                                 accum_out=s2)
            m2 = work.tile([P, 1], f32, tag="m2", name=f"m2_{it}")
            nc.vector.tensor_scalar(out=m2, in0=s1, scalar1=1.0 / D,
                                    scalar2=None, op0=alu.mult)
            msq = work.tile([P, 1], f32, tag="msq", name=f"msq_{it}")
            nc.vector.tensor_tensor(out=msq, in0=m2, in1=m2, op=alu.mult)
            var2 = work.tile([P, 1], f32, tag="var2", name=f"var2_{it}")
            nc.vector.scalar_tensor_tensor(out=var2, in0=s2, scalar=1.0 / D,
                                           in1=msq, op0=alu.mult,
                                           op1=alu.subtract)
            inv2 = _rsqrt_chain(nc, small, var2, EPS)
            st.update(od=od, m2=m2, inv2=inv2)

        def stage_c(st, it):
            """stylized LN2 + silu + transpose"""
            c1 = work.tile([P, D], bf16, tag="c1", name=f"c1_{it}")
            nc.vector.scalar_tensor_tensor(out=c1, in0=st["od"], scalar=st["m2"],
                                           in1=scale_rep, op0=alu.subtract,
                                           op1=alu.mult)
            y1 = work.tile([P, D], bf16, tag="y1", name=f"y1_{it}")
            nc.vector.scalar_tensor_tensor(out=y1, in0=c1, scalar=st["inv2"],
                                           in1=shift_rep, op0=alu.mult,
                                           op1=alu.add)
            th = work.tile([P, D], bf16, tag="th", name=f"th_{it}")
            nc.scalar.activation(out=th, in_=y1, func=act.Tanh, scale=0.5)
            sh = work.tile([P, D], bf16, tag="sh", name=f"sh_{it}")
            nc.vector.scalar_tensor_tensor(out=sh, in0=th, scalar=1.0, in1=y1,
                                           op0=alu.add, op1=alu.mult)
            shT = work.tile([P, KC, P], bf16, tag="shT", name=f"shT_{it}")
            for j in range(2):
                ps_t = ps_t_p.tile([P, 2, P], bf16, tag="pst",
                                   name=f"pst_{it}_{j}")
                for k in range(2):
                    c = 2 * j + k
                    nc.tensor.transpose(ps_t[:, k, :],
                                        in_=sh[:, c * P : (c + 1) * P],
                                        identity=ident)
                if j == 0:
                    nc.vector.tensor_copy(out=shT[:, 0:2, :], in_=ps_t)
                else:
                    nc.scalar.copy(out=shT[:, 2:4, :], in_=ps_t)
            st["shT"] = shT

        def stage_d(st, it):
            """out projection + residual + store"""
            r0 = it * P
            ps_y = ps_y_p.tile([P, D], f32, tag="psy", name=f"psy_{it}")
            for c in range(KC):
                nc.tensor.matmul(ps_y, lhsT=st["shT"][:, c, :], rhs=wo_sb[:, c, :],
                                 start=(c == 0), stop=(c == KC - 1))
            y_out = stream.tile([P, D], bf16, tag="y_out", name=f"y_{it}")
            nc.vector.tensor_tensor(out=y_out, in0=ps_y, in1=st["x"], op=alu.add)
            nc.scalar.dma_start(out=y_d[r0 : r0 + P, :], in_=y_out)

        with rep_cm:
            # software-pipelined: A(i) B1(i-1) B2(i-2) C(i-3) D(i-4)
            states = {}
            for step in range(TT + 4):
                if step < TT:
                    states[step] = stage_a(step)
                if 0 <= step - 1 < TT:
                    stage_b1(states[step - 1], step - 1)
                if 0 <= step - 2 < TT:
                    stage_b2(states[step - 2], step - 2)
                if 0 <= step - 3 < TT:
                    stage_c(states[step - 3], step - 3)
                if 0 <= step - 4 < TT:
                    stage_d(states[step - 4], step - 4)
                    del states[step - 4]

    if not nc.is_finalized():
        nc.finalize()
    return nc


def _prep_host(inputs):
    """Weight folding on host (numpy). Returns per-core input maps."""
    f32 = np.float32
    x = np.asarray(inputs["x"], f32)
    xf = np.asarray(inputs["xf"], f32)
    emb = np.asarray(inputs["emb"], f32)

    g_x = np.asarray(inputs["ln_x_g"], f32)
    b_x = np.asarray(inputs["ln_x_b"], f32)
    g_t = np.asarray(inputs["ln_t_g"], f32)
    b_t = np.asarray(inputs["ln_t_b"], f32)
    g_o = np.asarray(inputs["ln_o_g"], f32)
    b_o = np.asarray(inputs["ln_o_b"], f32)
    Wq = np.asarray(inputs["Wq"], f32)
    bq = np.asarray(inputs["bq"], f32)
    Wk = np.asarray(inputs["Wk"], f32)
    bk = np.asarray(inputs["bk"], f32)
    Wv = np.asarray(inputs["Wv"], f32)
    bv = np.asarray(inputs["bv"], f32)
    emb_W = np.asarray(inputs["emb_W"], f32)
    emb_b = np.asarray(inputs["emb_b"], f32)
    out_W = np.asarray(inputs["out_W"], f32)
    out_b = np.asarray(inputs["out_b"], f32)

    wq_eff = (g_x[:, None] * Wq).astype(BF16)
    bq_eff = b_x @ Wq + bq
    wk_eff = (g_t[:, None] * Wk).astype(BF16)
    bk_eff = b_t @ Wk + bk
    wv_eff = (g_t[:, None] * Wv).astype(BF16)
    bv_eff = b_t @ Wv + bv
    wo_eff = (0.5 * out_W).astype(BF16)
    wemb_eff = (0.5 * emb_W).astype(BF16)

    assert np.all(bq_eff == 0) and np.all(bk_eff == 0) and np.all(bv_eff == 0) \
        and np.all(out_b == 0), (
        "nonzero projection biases not emitted in this build")

    in_maps = []
    for b in range(B):
        in_maps.append({
            "x": np.ascontiguousarray(x[b]).astype(BF16),
            "xf": np.ascontiguousarray(xf[b]),
            "embt": np.ascontiguousarray(emb[b].reshape(TE // P, P).T),
            "wq": wq_eff, "wk": wk_eff, "wv": wv_eff, "wo": wo_eff,
            "wemb": wemb_eff,
            "go": g_o.reshape(1, D),
            "bo": b_o.reshape(1, D),
            "embb": emb_b.reshape(1, 2 * D),
        })
    return in_maps


_CACHED_NC = None


def kernel(**inputs) -> np.ndarray:
    global _CACHED_NC
    from concourse.bass_utils import run_bass_kernel_spmd

    in_maps = _prep_host(inputs)
    if _CACHED_NC is None:
        _CACHED_NC = build_program()
    res = run_bass_kernel_spmd(_CACHED_NC, in_maps, list(range(B)))
    out = np.stack([res.results[i]["y"] for i in range(B)]).astype(np.float32)
    return out


if __name__ == "__main__":
    import reference

    inputs = {k: np.asarray(v) for k, v in reference.setup_inputs().items()}
    y = kernel(**inputs)
    print("out", y.shape, y.dtype)



# revision 31
# speedup vs baseline: 1.0587x; 1.0587x over previous
"""Trainium2 Bass kernel for nn_CA_80461917323389 (sparse_attention).

Reference computation (per batch b, one NeuronCore per batch):
  xt  = LN(xf)                                   [N=256, TXT=768]
  q   = softmax((LN(x) @ Wq + bq).view(T,H,64))  [T=8192, H=8, 64]
  k   = softmax((xt @ Wk + bk).view(N,H,64))
  v   = (xt @ Wv + bv).view(N,H,64)
  attn[h] = k[:,h,:].T @ v[:,h,:]                [H, 64, 64]
  out = q @ attn (per head)                      [T, 512]
  eo  = silu(emb) @ emb_W + emb_b ; scale, shift = split(eo)
  h   = LN(out) * (1+scale) + shift
  y   = x + silu(h) @ out_W + out_b

Sharding: data-parallel over B=8 across the 8 cores.

Host-side prep is weights-only folding:
  - LN gains/biases folded into Wq/Wk/Wv (g[:,None]*W, b@W+bias)
  - silu(z) = (tanh(z/2)+1) * z * 0.5 -> the 0.5 is folded into out_W and
    emb_W so ScalarE only ever needs the exp_and_others table set (exp+tanh).
"""

import os
import sys

import numpy as np

sys.path.insert(0, "/opt/trn_rl_repo")

import ml_dtypes  # noqa: E402

BF16 = ml_dtypes.bfloat16

B, T, N, D, TXT, TE, H = 8, 8192, 256, 512, 768, 2048, 8
DH = D // H  # 64
P = 128
KC = D // P    # 4 k-chunks for D
KCT = TXT // P  # 6 k-chunks for TXT
EPS = 1e-5
RSQRT_MAGIC = 0x5F3759DF


def _rsqrt_chain(nc, pool, var_ap, eps, n_newton=1):
    """1/sqrt(var + eps) on VectorE only (no ACT table dependency).

    Quake-III bit trick init + Newton iterations. All ops on [P, w] tiles.
    """
    import concourse.mybir as mybir

    shape = list(var_ap.shape)
    alu = mybir.AluOpType
    vp = pool.tile(shape, mybir.dt.float32, tag="ch_vp")
    nc.vector.tensor_scalar(out=vp, in0=var_ap, scalar1=float(eps), scalar2=None,
                            op0=alu.add)
    y = pool.tile(shape, mybir.dt.float32, tag="ch_y")
    vi = vp.bitcast(mybir.dt.int32)
    yi = y.bitcast(mybir.dt.int32)
    # yi = MAGIC - (vi >> 1)
    nc.vector.tensor_scalar(out=yi, in0=vi, scalar1=1, scalar2=None,
                            op0=alu.logical_shift_right)
    nc.vector.tensor_scalar(out=yi, in0=yi, scalar1=-1, scalar2=RSQRT_MAGIC,
                            op0=alu.mult, op1=alu.add)
    t1 = pool.tile(shape, mybir.dt.float32, tag="ch_t1")
    for _ in range(n_newton):
        # y <- y * (1.5 - 0.5 * vp * y*y)
        nc.vector.tensor_tensor(out=t1, in0=y, in1=y, op=alu.mult)
        nc.vector.tensor_tensor(out=t1, in0=t1, in1=vp, op=alu.mult)
        nc.vector.tensor_scalar(out=t1, in0=t1, scalar1=-0.5, scalar2=1.5,
                                op0=alu.mult, op1=alu.add)
        nc.vector.tensor_tensor(out=y, in0=y, in1=t1, op=alu.mult)
    return y


def build_program(n_token_tiles=T // P, repeat=1):
    """Build the Bass program (shared by all 8 cores, SPMD).

    repeat>1 wraps the main token loop in a device-side For_i that redoes
    the identical work `repeat` times — used only for timing amplification.
    """
    import contextlib
    from contextlib import ExitStack

    import concourse.bacc as bacc
    import concourse.bass as bass
    import concourse.mybir as mybir
    import concourse.tile as tile
    from concourse.masks import make_identity

    f32 = mybir.dt.float32
    bf16 = mybir.dt.bfloat16
    alu = mybir.AluOpType
    act = mybir.ActivationFunctionType

    TT = n_token_tiles  # token tiles of 128 rows

    nc = bacc.Bacc("TRN2", target_bir_lowering=False, debug=False)
    x_d = nc.dram_tensor("x", [TT * P, D], bf16, kind="ExternalInput")
    xf_d = nc.dram_tensor("xf", [N, TXT], f32, kind="ExternalInput")
    embt_d = nc.dram_tensor("embt", [P, TE // P], f32, kind="ExternalInput")
    wq_d = nc.dram_tensor("wq", [D, D], bf16, kind="ExternalInput")
    wk_d = nc.dram_tensor("wk", [TXT, D], bf16, kind="ExternalInput")
    wv_d = nc.dram_tensor("wv", [TXT, D], bf16, kind="ExternalInput")
    wo_d = nc.dram_tensor("wo", [D, D], bf16, kind="ExternalInput")
    wemb_d = nc.dram_tensor("wemb", [TE, 2 * D], bf16, kind="ExternalInput")
    go_d = nc.dram_tensor("go", [1, D], f32, kind="ExternalInput")
    bo_d = nc.dram_tensor("bo", [1, D], f32, kind="ExternalInput")
    embb_d = nc.dram_tensor("embb", [1, 2 * D], f32, kind="ExternalInput")
    y_d = nc.dram_tensor("y", [TT * P, D], bf16, kind="ExternalOutput")

    with tile.TileContext(nc) as tc, ExitStack() as ctx:
        const = ctx.enter_context(tc.tile_pool(name="const", bufs=1))

        # ---- constants / weights into SBUF ----
        ident = const.tile([P, P], bf16)
        make_identity(nc, ident)
        ones_f32 = const.tile([1, P], f32)
        nc.vector.memset(ones_f32, 1.0)

        wq_sb = const.tile([P, KC, D], bf16)
        nc.sync.dma_start(out=wq_sb, in_=wq_d.rearrange("(c p) n -> p c n", p=P))
        wk_sb = const.tile([P, KCT, D], bf16)
        nc.sync.dma_start(out=wk_sb, in_=wk_d.rearrange("(c p) n -> p c n", p=P))
        wv_sb = const.tile([P, KCT, D], bf16)
        nc.sync.dma_start(out=wv_sb, in_=wv_d.rearrange("(c p) n -> p c n", p=P))
        wo_sb = const.tile([P, KC, D], bf16)
        nc.sync.dma_start(out=wo_sb, in_=wo_d.rearrange("(c p) n -> p c n", p=P))
        wemb_sb = const.tile([P, TE // P, 2 * D], bf16)
        nc.sync.dma_start(out=wemb_sb, in_=wemb_d.rearrange("(c p) n -> p c n", p=P))
        go_sb = const.tile([1, D], f32)
        nc.sync.dma_start(out=go_sb, in_=go_d[:, :])
        bo_sb = const.tile([1, D], f32)
        nc.sync.dma_start(out=bo_sb, in_=bo_d[:, :])
        embb_sb = const.tile([1, 2 * D], f32)
        nc.sync.dma_start(out=embb_sb, in_=embb_d[:, :])

        scale_rep = const.tile([P, D], bf16)   # (1+scale)*g_o, replicated
        shift_rep = const.tile([P, D], bf16)   # b_o*(1+scale)+shift, replicated
        a_sb = const.tile([P, KC, DH * 2 + 2], bf16)  # head-pair blockdiag + sum cols

        small = ctx.enter_context(tc.tile_pool(name="small", bufs=int(os.environ.get("KBUF_SMALL", 12))))

        # =================== prologue: eo -> scale/shift ===================
        with tc.tile_pool(name="pro_eo", bufs=2) as pro, \
             tc.tile_pool(name="pro_eo_ps", bufs=1, space="PSUM") as pro_ps:
            embt = pro.tile([P, TE // P], f32)
            nc.sync.dma_start(out=embt, in_=embt_d[:, :])
            th_e = pro.tile([P, TE // P], f32)
            nc.scalar.activation(out=th_e, in_=embt, func=act.Tanh, scale=0.5)
            se = pro.tile([P, TE // P], bf16)
            th_p1 = pro.tile([P, TE // P], f32)
            nc.vector.tensor_scalar(out=th_p1, in0=th_e, scalar1=1.0,
                                    scalar2=None, op0=alu.add)
            nc.vector.tensor_tensor(out=se, in0=th_p1, in1=embt, op=alu.mult)
            ps_sc = pro_ps.tile([1, D], f32)
            ps_sh = pro_ps.tile([1, D], f32)
            nkc = TE // P
            for kc in range(nkc):
                nc.tensor.matmul(ps_sc, lhsT=se[:, kc : kc + 1],
                                 rhs=wemb_sb[:, kc, 0:D],
                                 start=(kc == 0), stop=(kc == nkc - 1))
            for kc in range(nkc):
                nc.tensor.matmul(ps_sh, lhsT=se[:, kc : kc + 1],
                                 rhs=wemb_sb[:, kc, D : 2 * D],
                                 start=(kc == 0), stop=(kc == nkc - 1))
            # sp1 = (scale + emb_b[:D]) + 1
            sp1 = pro.tile([1, D], f32)
            nc.vector.scalar_tensor_tensor(out=sp1, in0=ps_sc, scalar=1.0,
                                           in1=embb_sb[:, 0:D],
                                           op0=alu.add, op1=alu.add)
            scale_row = pro.tile([1, D], f32)
            nc.vector.tensor_tensor(out=scale_row, in0=sp1, in1=go_sb, op=alu.mult)
            # shift_row = (shift + emb_b[D:]) + b_o * sp1
            t_bo = pro.tile([1, D], f32)
            nc.vector.tensor_tensor(out=t_bo, in0=sp1, in1=bo_sb, op=alu.mult)
            shift_row = pro.tile([1, D], f32)
            nc.vector.scalar_tensor_tensor(out=shift_row, in0=ps_sh, scalar=0.0,
                                           in1=embb_sb[:, D : 2 * D],
                                           op0=alu.add, op1=alu.add)
            nc.vector.tensor_tensor(out=shift_row, in0=shift_row, in1=t_bo,
                                    op=alu.add)
            # broadcast rows across 128 partitions via ones-matmul
            ps_bc = pro_ps.tile([P, D], f32, tag="bc")
            nc.tensor.matmul(ps_bc, lhsT=ones_f32, rhs=scale_row,
                             start=True, stop=True)
            nc.scalar.copy(out=scale_rep, in_=ps_bc)
            ps_bc2 = pro_ps.tile([P, D], f32, tag="bc")
            nc.tensor.matmul(ps_bc2, lhsT=ones_f32, rhs=shift_row,
                             start=True, stop=True)
            nc.scalar.copy(out=shift_rep, in_=ps_bc2)

        # =================== prologue: k/v -> attn pairs ===================
        with tc.tile_pool(name="pro_kv", bufs=2) as kvp, \
             tc.tile_pool(name="pro_kv_ps", bufs=1, space="PSUM") as kv_ps, \
             tc.tile_pool(name="pro_a_ps", bufs=4, space="PSUM") as a_ps:
            NTILES = N // P  # 2
            k_n = [None] * NTILES
            v_b = [None] * NTILES
            for tt in range(NTILES):
                xf_sb = kvp.tile([P, TXT], f32, tag="xf")
                nc.sync.dma_start(out=xf_sb, in_=xf_d[tt * P : (tt + 1) * P, :])
                st = kvp.tile([P, 3, 6], f32, tag="st")
                xf_g = xf_sb.rearrange("p (g d) -> p g d", g=3)
                for g in range(3):
                    nc.vector.bn_stats(out=st[:, g, :], in_=xf_g[:, g, :])
                mv = kvp.tile([P, 2], f32, tag="mv")
                nc.vector.bn_aggr(out=mv, in_=st)
                inv_t = _rsqrt_chain(nc, small, mv[:, 1:2], EPS)
                xtn = kvp.tile([P, TXT], bf16, tag="xtn")
                nc.vector.tensor_scalar(out=xtn, in0=xf_sb, scalar1=mv[:, 0:1],
                                        scalar2=inv_t, op0=alu.subtract,
                                        op1=alu.mult)
                xtT = kvp.tile([P, KCT, P], bf16, tag="xtT")
                nc.sync.dma_start_transpose(out=xtT, in_=xtn)

                ps_k = kv_ps.tile([P, D], f32, tag="psk")
                for c in range(KCT):
                    nc.tensor.matmul(ps_k, lhsT=xtT[:, c, :], rhs=wk_sb[:, c, :],
                                     start=(c == 0), stop=(c == KCT - 1))
                k_e = kvp.tile([P, D], bf16, tag="ke")
                nc.scalar.activation(out=k_e, in_=ps_k, func=act.Exp)
                ks = kvp.tile([P, H], f32, tag="ks")
                nc.vector.tensor_reduce(out=ks, in_=k_e.rearrange(
                    "p (h d) -> p h d", h=H), axis=mybir.AxisListType.X,
                    op=alu.add)
                kr = kvp.tile([P, H], f32, tag="kr")
                nc.vector.reciprocal(out=kr, in_=ks)
                k_n[tt] = kvp.tile([P, D], bf16, tag=f"kn{tt}", name=f"kn{tt}")
                nc.vector.tensor_tensor(
                    out=k_n[tt].rearrange("p (h d) -> p h d", h=H),
                    in0=k_e.rearrange("p (h d) -> p h d", h=H),
                    in1=kr.unsqueeze(2).broadcast_to([P, H, DH]), op=alu.mult)

                ps_v = kv_ps.tile([P, D], f32, tag="psv")
                for c in range(KCT):
                    nc.tensor.matmul(ps_v, lhsT=xtT[:, c, :], rhs=wv_sb[:, c, :],
                                     start=(c == 0), stop=(c == KCT - 1))
                v_b[tt] = kvp.tile([P, D], bf16, tag=f"vb{tt}", name=f"vb{tt}")
                nc.scalar.copy(out=v_b[tt], in_=ps_v)

            # attn[h] = k[:,h].T @ v[:,h], assembled as head-pair blockdiag
            nc.vector.memset(a_sb, 0.0)
            for c in range(KC):
                ps_a = a_ps.tile([P, P], f32)
                for tt in range(NTILES):
                    h0 = 2 * c
                    nc.tensor.matmul(
                        ps_a[0:DH, 0:DH],
                        lhsT=k_n[tt][:, h0 * DH : (h0 + 1) * DH],
                        rhs=v_b[tt][:, h0 * DH : (h0 + 1) * DH],
                        start=(tt == 0), stop=(tt == NTILES - 1))
                for tt in range(NTILES):
                    h1 = 2 * c + 1
                    nc.tensor.matmul(
                        ps_a[DH : 2 * DH, DH : 2 * DH],
                        lhsT=k_n[tt][:, h1 * DH : (h1 + 1) * DH],
                        rhs=v_b[tt][:, h1 * DH : (h1 + 1) * DH],
                        start=(tt == 0), stop=(tt == NTILES - 1),
                        tile_position=(0, 64))
                nc.vector.tensor_copy(out=a_sb[0:DH, c, 0:DH],
                                      in_=ps_a[0:DH, 0:DH])
                nc.vector.tensor_copy(out=a_sb[DH : 2 * DH, c, DH : 2 * DH],
                                      in_=ps_a[DH : 2 * DH, DH : 2 * DH])
            nc.vector.memset(a_sb[0:DH, :, 2 * DH : 2 * DH + 1], 1.0)
            nc.vector.memset(a_sb[DH : 2 * DH, :, 2 * DH + 1 : 2 * DH + 2], 1.0)

        # =================== main loop over token tiles ===================
        stream = ctx.enter_context(tc.tile_pool(name="stream", bufs=int(os.environ.get("KBUF_STREAM", 7))))
        work = ctx.enter_context(tc.tile_pool(name="work", bufs=int(os.environ.get("KBUF_WORK", 7))))
        ps_qT_p = ctx.enter_context(tc.tile_pool(name="ps_qT", bufs=2, space="PSUM"))
        ps_o_p = ctx.enter_context(tc.tile_pool(name="ps_o", bufs=2, space="PSUM"))
        ps_s_p = ctx.enter_context(tc.tile_pool(name="ps_s", bufs=1, space="PSUM"))
        ps_y_p = ctx.enter_context(tc.tile_pool(name="ps_y", bufs=1, space="PSUM"))
        ps_t_p = ctx.enter_context(tc.tile_pool(name="ps_t", bufs=2, space="PSUM"))

        rep_cm = tc.For_i(0, repeat, 1) if repeat > 1 else contextlib.nullcontext()

        def stage_a(it):
            """load + LN1 + transpose -> returns state for stage b"""
            r0 = it * P
            xt_f = stream.tile([P, D], bf16, tag="x_in", name=f"x_{it}")
            nc.sync.dma_start(out=xt_f, in_=x_d[r0 : r0 + P, :])
            st6 = work.tile([P, 6], f32, tag="st6", name=f"st6_{it}")
            nc.vector.bn_stats(out=st6, in_=xt_f)
            mv = work.tile([P, 2], f32, tag="mv", name=f"mv_{it}")
            nc.vector.bn_aggr(out=mv, in_=st6)
            inv1 = _rsqrt_chain(nc, small, mv[:, 1:2], EPS)
            nmi = work.tile([P, 1], f32, tag="nmi", name=f"nmi_{it}")
            nc.vector.tensor_scalar(out=nmi, in0=mv[:, 0:1], scalar1=inv1,
                                    scalar2=-1.0, op0=alu.mult, op1=alu.mult)
            xn_b = work.tile([P, D], bf16, tag="xn", name=f"xn_{it}")
            nc.scalar.activation(out=xn_b, in_=xt_f,
                                 func=act.Identity, scale=inv1, bias=nmi)
            xT = work.tile([P, KC, P], bf16, tag="xT", name=f"xT_{it}")
            nc.sync.dma_start_transpose(out=xT, in_=xn_b)
            return {"x": xt_f, "xT": xT}

        def stage_b1(st, it):
            """q projection + exp"""
            ps_qT = ps_qT_p.tile([P, KC, P], f32, tag="psqT", name=f"psqT_{it}")
            for dc in range(KC):
                for kc in range(KC):
                    nc.tensor.matmul(ps_qT[:, dc, :],
                                     lhsT=wq_sb[:, kc, dc * P : (dc + 1) * P],
                                     rhs=st["xT"][:, kc, :],
                                     start=(kc == 0), stop=(kc == KC - 1))
            q_eT = work.tile([P, KC, P], bf16, tag="qeT", name=f"qeT_{it}")
            nc.scalar.activation(out=q_eT, in_=ps_qT, func=act.Exp)
            st["qeT"] = q_eT

        def stage_b2(st, it):
            """attention apply + softmax div + LN2 scalars"""
            q_eT = st.pop("qeT")
            ps_o = ps_o_p.tile([P, D], f32, tag="pso", name=f"pso_{it}")
            ps_s = ps_s_p.tile([P, H], f32, tag="pss", name=f"pss_{it}")
            for c in range(KC):
                nc.tensor.matmul(ps_o[:, c * P : (c + 1) * P],
                                 lhsT=q_eT[:, c, :], rhs=a_sb[:, c, 0 : 2 * DH],
                                 start=True, stop=True)
                nc.tensor.matmul(ps_s[:, 2 * c : 2 * c + 2],
                                 lhsT=q_eT[:, c, :],
                                 rhs=a_sb[:, c, 2 * DH : 2 * DH + 2],
                                 start=True, stop=True)
            r = work.tile([P, H], f32, tag="r", name=f"r_{it}")
            nc.vector.reciprocal(out=r, in_=ps_s)
            od = work.tile([P, D], bf16, tag="od", name=f"od_{it}")
            s1 = work.tile([P, 1], f32, tag="s1", name=f"s1_{it}")
            nc.vector.scalar_tensor_tensor(
                out=od.rearrange("p (h d) -> p h d", h=H),
                in0=ps_o.rearrange("p (h d) -> p h d", h=H), scalar=1.0,
                in1=r.unsqueeze(2).broadcast_to([P, H, DH]),
                op0=alu.mult, op1=alu.mult, accum_out=s1)
            junk = work.tile([P, D], bf16, tag="junk", name=f"junk_{it}")
            s2 = work.tile([P, 1], f32, tag="s2", name=f"s2_{it}")
            nc.scalar.activation(out=junk, in_=od, func=act.Square,
                                 accum_out=s2)
            m2 = work.tile([P, 1], f32, tag="m2", name=f"m2_{it}")
            nc.vector.tensor_scalar(out=m2, in0=s1, scalar1=1.0 / D,
                                    scalar2=None, op0=alu.mult)
            msq = work.tile([P, 1], f32, tag="msq", name=f"msq_{it}")
            nc.vector.tensor_tensor(out=msq, in0=m2, in1=m2, op=alu.mult)
            var2 = work.tile([P, 1], f32, tag="var2", name=f"var2_{it}")
            nc.vector.scalar_tensor_tensor(out=var2, in0=s2, scalar=1.0 / D,
                                           in1=msq, op0=alu.mult,
                                           op1=alu.subtract)
            inv2 = _rsqrt_chain(nc, small, var2, EPS)
            st.update(od=od, m2=m2, inv2=inv2)

        def stage_c(st, it):
            """stylized LN2 + silu + transpose"""
            c1 = work.tile([P, D], bf16, tag="c1", name=f"c1_{it}")
            nc.vector.scalar_tensor_tensor(out=c1, in0=st["od"], scalar=st["m2"],
                                           in1=scale_rep, op0=alu.subtract,
                                           op1=alu.mult)
            y1 = work.tile([P, D], bf16, tag="y1", name=f"y1_{it}")
            nc.vector.scalar_tensor_tensor(out=y1, in0=c1, scalar=st["inv2"],
                                           in1=shift_rep, op0=alu.mult,
                                           op1=alu.add)
            th = work.tile([P, D], bf16, tag="th", name=f"th_{it}")
            nc.scalar.activation(out=th, in_=y1, func=act.Tanh, scale=0.5)
            sh = work.tile([P, D], bf16, tag="sh", name=f"sh_{it}")
            nc.vector.scalar_tensor_tensor(out=sh, in0=th, scalar=1.0, in1=y1,
                                           op0=alu.add, op1=alu.mult)
            shT = work.tile([P, KC, P], bf16, tag="shT", name=f"shT_{it}")
            for j in range(2):
                ps_t = ps_t_p.tile([P, 2, P], bf16, tag="pst",
                                   name=f"pst_{it}_{j}")
                for k in range(2):
                    c = 2 * j + k
                    nc.tensor.transpose(ps_t[:, k, :],
                                        in_=sh[:, c * P : (c + 1) * P],
                                        identity=ident)
                if j == 0:
                    nc.vector.tensor_copy(out=shT[:, 0:2, :], in_=ps_t)
                else:
                    nc.scalar.copy(out=shT[:, 2:4, :], in_=ps_t)
            st["shT"] = shT

        def stage_d(st, it):
            """out projection + residual + store"""
            r0 = it * P
            ps_y = ps_y_p.tile([P, D], f32, tag="psy", name=f"psy_{it}")
            for c in range(KC):
                nc.tensor.matmul(ps_y, lhsT=st["shT"][:, c, :], rhs=wo_sb[:, c, :],
                                 start=(c == 0), stop=(c == KC - 1))
            y_out = stream.tile([P, D], bf16, tag="y_out", name=f"y_{it}")
            nc.vector.tensor_tensor(out=y_out, in0=ps_y, in1=st["x"], op=alu.add)
            nc.scalar.dma_start(out=y_d[r0 : r0 + P, :], in_=y_out)

        with rep_cm:
            # software-pipelined: A(i) B1(i-1) B2(i-2) C(i-3) D(i-4)
            states = {}
            for step in range(TT + 4):
                if step < TT:
                    states[step] = stage_a(step)
                if 0 <= step - 1 < TT:
                    stage_b1(states[step - 1], step - 1)
                if 0 <= step - 2 < TT:
                    stage_b2(states[step - 2], step - 2)
                if 0 <= step - 3 < TT:
                    stage_c(states[step - 3], step - 3)
                if 0 <= step - 4 < TT:
                    stage_d(states[step - 4], step - 4)
                    del states[step - 4]

    if not nc.is_finalized():
        nc.finalize()
    return nc


def _prep_host(inputs):
    """Weight folding on host (numpy). Returns per-core input maps."""
    f32 = np.float32
    x = np.asarray(inputs["x"], f32)
    xf = np.asarray(inputs["xf"], f32)
    emb = np.asarray(inputs["emb"], f32)

    g_x = np.asarray(inputs["ln_x_g"], f32)
    b_x = np.asarray(inputs["ln_x_b"], f32)
    g_t = np.asarray(inputs["ln_t_g"], f32)
    b_t = np.asarray(inputs["ln_t_b"], f32)
    g_o = np.asarray(inputs["ln_o_g"], f32)
    b_o = np.asarray(inputs["ln_o_b"], f32)
    Wq = np.asarray(inputs["Wq"], f32)
    bq = np.asarray(inputs["bq"], f32)
    Wk = np.asarray(inputs["Wk"], f32)
    bk = np.asarray(inputs["bk"], f32)
    Wv = np.asarray(inputs["Wv"], f32)
    bv = np.asarray(inputs["bv"], f32)
    emb_W = np.asarray(inputs["emb_W"], f32)
    emb_b = np.asarray(inputs["emb_b"], f32)
    out_W = np.asarray(inputs["out_W"], f32)
    out_b = np.asarray(inputs["out_b"], f32)

    wq_eff = (g_x[:, None] * Wq).astype(BF16)
    bq_eff = b_x @ Wq + bq
    wk_eff = (g_t[:, None] * Wk).astype(BF16)
    bk_eff = b_t @ Wk + bk
    wv_eff = (g_t[:, None] * Wv).astype(BF16)
    bv_eff = b_t @ Wv + bv
    wo_eff = (0.5 * out_W).astype(BF16)
    wemb_eff = (0.5 * emb_W).astype(BF16)

    assert np.all(bq_eff == 0) and np.all(bk_eff == 0) and np.all(bv_eff == 0) \
        and np.all(out_b == 0), (
        "nonzero projection biases not emitted in this build")

    in_maps = []
    for b in range(B):
        in_maps.append({
            "x": np.ascontiguousarray(x[b]).astype(BF16),
            "xf": np.ascontiguousarray(xf[b]),
            "embt": np.ascontiguousarray(emb[b].reshape(TE // P, P).T),
            "wq": wq_eff, "wk": wk_eff, "wv": wv_eff, "wo": wo_eff,
            "wemb": wemb_eff,
            "go": g_o.reshape(1, D),
            "bo": b_o.reshape(1, D),
            "embb": emb_b.reshape(1, 2 * D),
        })
    return in_maps


_CACHED_NC = None


def kernel(**inputs) -> np.ndarray:
    global _CACHED_NC
    from concourse.bass_utils import run_bass_kernel_spmd

    in_maps = _prep_host(inputs)
    if _CACHED_NC is None:
        _CACHED_NC = build_program()
    res = run_bass_kernel_spmd(_CACHED_NC, in_maps, list(range(B)))
    out = np.stack([res.results[i]["y"] for i in range(B)]).astype(np.float32)
    return out


if __name__ == "__main__":
    import reference

    inputs = {k: np.asarray(v) for k, v in reference.setup_inputs().items()}
    y = kernel(**inputs)
    print("out", y.shape, y.dtype)

